# revision 82
# baseline (speedup 1.0000x reference)
"""Trainium2 Bass kernel for nn_BertSelfAttention_7962869367489.

Dual-branch (self + cross/"knowledge") BERT attention, B=4, S=1024, K=512,
H=1024, NH=16, HD=64, fp32 in/out.

Sharding: 8 cores = (batch b in 0..3) x (head-group hg in 0..1, 8 heads each).
All six projections column-split by head-group; per-head attention core-local;
output columns disjoint per core (pure concatenation, no collectives).

All PE math in bf16 (1 cycle/row in the cost model, rel err ~5e-3 vs the 2e-2
gate). The host pre-transposes hs/ehs into [p, hc, s] bf16 layouts and
pre-quantizes/pre-tiles all weights into their exact SBUF layouts, so the
device does zero transposes and minimal DMA (9.5 MB in, 2 MB out per core).

Per-core pipeline:
  1. Projections (bf16): QT/KT/KQT/KKT = W.T @ hsT (transposed outputs, PSUM
     -> SBUF bf16 with bias via DVE tensor_scalar; gpsimd has no PSUM port);
     V/KV = hsT.T @ Wv in natural [t, j] orientation.
  2. Per head h, per key-tile kt: scoresT[t, s] (PSUM f32, 2x512 free),
     exp on ACT with per-partition mask bias and 1/8 scale -> E bf16 (all 12
     E tiles of a head stay live in a 20-deep SBUF ring).
  3. Once a branch's exps are ~6 units old, its ctx block runs: per s-tile,
     one PSUM accumulation series over kt: ctx[s, d] += E_slice.T @ V_kt
     (free 64) and den[s] += E_slice.T @ ones2 (free 1, same stationary;
     ones2 = 2.0 folds the (ctx+kctx)/2 average into the normalization).
     Series run strictly one-at-a-time per bank: a matmul's start= clears
     has_written for the WHOLE bank, so interleaved series lose partials.
  4. post: reciprocal of den, fused PSUM-read normalize per branch, add on
     DVE/Pool into [s, h, d] staging; output DMA'd per head.
  Knowledge branch runs skewed 2 heads behind self; projection chunks are
  paced between attention units ahead of their consumers' slots; dummy
  warmup matmuls during the DMA prelude hold the PE p-state ramp.
"""
import numpy as np
import ml_dtypes
from contextlib import ExitStack

import concourse.bacc as bacc
import concourse.tile as tile
import concourse.mybir as mybir
from concourse.bass_utils import run_bass_kernel_spmd

F32 = mybir.dt.float32
BF16 = mybir.dt.bfloat16
AF = mybir.ActivationFunctionType
ALU = mybir.AluOpType

P = 128
S = 1024        # query length
TKS = 1024      # self-branch key length
TKK = 512       # knowledge-branch key length
H = 1024        # model dim (projection contraction)
HG = 512        # per-core output width (8 heads x 64)
NHL = 8         # heads per core
HD = 64
HC = H // P     # 8 contraction chunks
NJT = HG // P   # 4 column tiles per projection
INV = 0.125     # 1/sqrt(64)

_CACHE = {}
_DUMP = False
_WARMUP = 8


def _build():
    nc = bacc.Bacc(target_bir_lowering=False, debug=False)

    hsT = [nc.dram_tensor(f"hsT{i}", [P, HC, S // 2], BF16, kind="ExternalInput")
           for i in range(2)]
    ehsT = nc.dram_tensor("ehsT", [P, HC, TKK], BF16, kind="ExternalInput")
    w_t = {nm: nc.dram_tensor(f"wt_{nm}", [NJT, P, HC, P], BF16,
                              kind="ExternalInput")
           for nm in ["q", "k", "kq", "kk"]}
    w_v = {nm: nc.dram_tensor(f"wv_{nm}", [P, HC, HG], BF16,
                              kind="ExternalInput")
           for nm in ["v", "kv"]}
    b_t = {nm: nc.dram_tensor(f"bt_{nm}", [P, NJT], F32, kind="ExternalInput")
           for nm in ["q", "k", "kq", "kk"]}
    b_v = {nm: nc.dram_tensor(f"bv_{nm}", [HG], F32, kind="ExternalInput")
           for nm in ["v", "kv"]}
    mask = nc.dram_tensor("mask", [P, TKS // P], F32, kind="ExternalInput")
    emask = nc.dram_tensor("emask", [P, TKK // P], F32, kind="ExternalInput")
    out = nc.dram_tensor("out", [S, HG], F32, kind="ExternalOutput")

    with tile.TileContext(nc) as tc, ExitStack() as ctx:
        const = ctx.enter_context(tc.tile_pool(name="const", bufs=1))
        persist = ctx.enter_context(tc.tile_pool(name="persist", bufs=1))
        wtp = ctx.enter_context(tc.tile_pool(name="wtp", bufs=16))
        epool = ctx.enter_context(tc.tile_pool(name="epool", bufs=18))
        cspool = ctx.enter_context(tc.tile_pool(name="cspool", bufs=2))
        # PSUM: 8 banks exactly
        psc = ctx.enter_context(tc.tile_pool(name="psc", bufs=2, space="PSUM"))
        psx = ctx.enter_context(tc.tile_pool(name="psx", bufs=1, space="PSUM"))
        psj = ctx.enter_context(tc.tile_pool(name="psj", bufs=1, space="PSUM"))

        # ---- constants (tiles; DMAs are issued in the SP load order below) --
        mask_sb = const.tile([P, TKS // P], F32)
        emask_sb = const.tile([P, TKK // P], F32)
        bias_col = {nm: const.tile([P, NJT], F32, name=f"bc_{nm}")
                    for nm in ["q", "k", "kq", "kk"]}
        bias_row = {nm: const.tile([P, HG], F32, name=f"br_{nm}")
                    for nm in ["v", "kv"]}
        wu = const.tile([P, 512], BF16)
        nc.vector.memset(wu, 0.02)
        ones2 = const.tile([P, 1], BF16)
        nc.vector.memset(ones2, 2.0)

        # ---- persistent activations ----
        hsTt = [persist.tile([P, HC, S // 2], BF16, name=f"hsTt{i}")
                for i in range(2)]
        ehsTt = persist.tile([P, HC, TKK], BF16)
        wvt = {}
        for nm in ["v", "kv"]:
            wvt[nm] = persist.tile([P, HC, HG], BF16, name=f"wvt_{nm}")
        QT = persist.tile([P, NJT, S], BF16)      # [j%128, jt, s]
        KT = persist.tile([P, NJT, TKS], BF16)
        KQT = persist.tile([P, NJT, S], BF16)
        KKT = persist.tile([P, NJT, TKK], BF16)
        Vsb = persist.tile([P, TKS // P, NHL, HD], BF16)   # [t%128, tt, h, d]
        KVsb = persist.tile([P, TKK // P, NHL, HD], BF16)
        # output staging per head-pair
        op_t = [persist.tile([P, S // P, 2, HD], F32, name=f"op{i}")
                for i in range(4)]

        # PSUM persistent-ish accumulators (bufs=1 pools, reused per head)
        ctxS = psx.tile([P, S // P, HD], F32, name="ctxS", tag="ctxS")
        ctxK = psx.tile([P, S // P, HD], F32, name="ctxK", tag="ctxK")
        den = psx.tile([P, 2, S // P], F32, name="den", tag="den")

        # ---- input DMAs (SP queue, consumption order) ----
        def load_wt(nm, jt):
            t = wtp.tile([P, HC, P], BF16, name=f"w_{nm}{jt}", tag="wt")
            nc.sync.dma_start(out=t, in_=w_t[nm][jt])
            return t

        wtiles = {}
        wtiles[("q", 0)] = load_wt("q", 0)
        nc.sync.dma_start(out=hsTt[0][:, 0:4, :], in_=hsT[0].ap()[:, 0:4, :])
        wtiles[("k", 0)] = load_wt("k", 0)
        nc.sync.dma_start(out=hsTt[0][:, 4:8, :], in_=hsT[0].ap()[:, 4:8, :])
        nc.sync.dma_start(out=hsTt[1][:, 0:4, :], in_=hsT[1].ap()[:, 0:4, :])
        nc.sync.dma_start(out=bias_col["q"], in_=b_t["q"].ap())
        nc.sync.dma_start(out=bias_col["k"], in_=b_t["k"].ap())
        nc.sync.dma_start(out=mask_sb, in_=mask.ap())
        nc.sync.dma_start(out=hsTt[1][:, 4:8, :], in_=hsT[1].ap()[:, 4:8, :])
        nc.sync.dma_start(out=wvt["v"], in_=w_v["v"].ap())
        nc.sync.dma_start(out=bias_row["v"], in_=b_v["v"].ap().unsqueeze(0)
                          .broadcast_to([P, HG]))
        nc.sync.dma_start(out=ehsTt, in_=ehsT.ap())
        nc.sync.dma_start(out=wvt["kv"], in_=w_v["kv"].ap())
        nc.sync.dma_start(out=bias_col["kq"], in_=b_t["kq"].ap())
        nc.sync.dma_start(out=bias_col["kk"], in_=b_t["kk"].ap())
        nc.sync.dma_start(out=emask_sb, in_=emask.ap())
        nc.sync.dma_start(out=bias_row["kv"], in_=b_v["kv"].ap().unsqueeze(0)
                          .broadcast_to([P, HG]))
        # prefetch every remaining weight tile now (transfers are 728ns each;
        # lazy loads were stalling their first consumer by ~1us)
        for jt in range(1, NJT):
            wtiles[("q", jt)] = load_wt("q", jt)
            wtiles[("k", jt)] = load_wt("k", jt)
        for jt in range(NJT):
            wtiles[("kq", jt)] = load_wt("kq", jt)
            wtiles[("kk", jt)] = load_wt("kk", jt)

        # remaining weight loads are issued lazily right before their first
        # consumer chunk is emitted (SP queue order == emission order)
        def get_wt(nm, jt):
            if (nm, jt) not in wtiles:
                wtiles[(nm, jt)] = load_wt(nm, jt)
            return wtiles[(nm, jt)]

        # ---- PE warmup (p-state ramp) during the DMA prelude ----
        def warmup(n):
            for _ in range(n):
                nc.tensor.matmul(ctxS.rearrange("p a b -> p (a b)"),
                                 lhsT=wu[:, 0:P], rhs=wu,
                                 start=True, stop=True)

        # ---- projection chunk emitters ----
        # PSUM-reading copies must be DVE or ACT (gpsimd has no PSUM port);
        # DVE's shallow queue keeps the single psj bank's WAR latency low
        tcopy_engine = {"q": "vector", "k": "vector",
                        "kq": "vector", "kk": "vector"}

        def proj_t_chunk(nm, dst, src_half, jt, sc, use_psc=False):
            # dst[:, jt, sc*512:(sc+1)*512] = (W[:, jt].T @ hsT)[:, sc] + b
            w = get_wt(nm, jt)
            src = ehsTt if nm == "kk" else src_half[sc]
            if use_psc:
                # prelude: the score pool is idle pre-attention; using it
                # pipelines the first chunks past the single psj bank
                ps = psc.tile([P, 2, 512], F32, name=f"pj_{nm}{jt}{sc}",
                              tag="st")[:, 0, :]
            else:
                ps = psj.tile([P, 512], F32, name=f"pj_{nm}{jt}{sc}", tag="pj")
            for hc in range(HC):
                nc.tensor.matmul(ps, lhsT=w[:, hc, :], rhs=src[:, hc, :],
                                 start=(hc == 0), stop=(hc == HC - 1))
            dsl = dst[:, jt, sc * 512:(sc + 1) * 512]
            eng = tcopy_engine[nm]
            if eng == "act":
                nc.scalar.add(dsl, ps, bias_col[nm][:, jt:jt + 1])
            else:
                e = nc.vector if eng == "vector" else nc.gpsimd
                e.tensor_scalar_add(dsl, ps, bias_col[nm][:, jt:jt + 1])

        def proj_v_chunk(nm, dst, tt):
            # dst[:, tt, :, :] = (hsT_chunk.T @ Wv) + bias_row  (natural [t, j])
            if nm == "v":
                src = hsTt[tt // 4][:, :, (tt % 4) * P:(tt % 4 + 1) * P]
            else:
                src = ehsTt[:, :, tt * P:(tt + 1) * P]
            ps = psj.tile([P, 512], F32, name=f"pv_{nm}{tt}", tag="pj")
            for hc in range(HC):
                nc.tensor.matmul(ps, lhsT=src[:, hc, :], rhs=wvt[nm][:, hc, :],
                                 start=(hc == 0), stop=(hc == HC - 1))
            nc.vector.scalar_tensor_tensor(
                out=dst[:, tt, :, :],
                in0=ps.rearrange("p (h d) -> p h d", h=NHL),
                scalar=1.0,
                in1=bias_row[nm].rearrange("p (h d) -> p h d", h=NHL),
                op0=ALU.mult, op1=ALU.add)

        # ---- attention emitters ----
        def score_unit(h, kt, knl):
            base = (h % 2) * HD
            jt = h // 2
            kmat, qmat, msk = ((KKT, KQT, emask_sb) if knl
                               else (KT, QT, mask_sb))
            st = psc.tile([P, 2, 512], F32,
                          name=f"st_{h}_{kt}_{int(knl)}", tag="st")
            for sc in range(2):
                nc.tensor.matmul(
                    st[:, sc, :],
                    lhsT=kmat[base:base + HD, jt, kt * P:(kt + 1) * P],
                    rhs=qmat[base:base + HD, jt, sc * 512:(sc + 1) * 512],
                    start=True, stop=True)
            e = epool.tile([P, 2, 512], BF16,
                           name=f"e_{h}_{kt}_{int(knl)}",
                           tag=("e_def" if h == 0 else "e"),
                           bufs=(12 if h == 0 else 18))
            nc.scalar.activation(e, st, AF.Exp,
                                 bias=msk[:, kt:kt + 1], scale=INV)
            if _DUMP and h == 0 and kt < 2:
                d = nc.dram_tensor(f"d_e_{kt}_{int(knl)}", [P, 2, 512], BF16,
                                   kind="ExternalOutput")
                nc.sync.dma_start(out=d.ap(), in_=e)
            return e

        def emit_ctx_block(h, knl, elist, nkt):
            # One PSUM accumulation series may be open per bank at a time
            # (matmul start= clears has_written for the WHOLE bank), so each
            # (branch, st) series runs to completion before the next opens.
            # ctx lives in its own bank, den in another: both can be open.
            cmat = ctxK if knl else ctxS
            vmat = KVsb if knl else Vsb
            efs = [e.rearrange("p a b -> p (a b)") for e in elist]
            if _DUMP and h in (0, 6) and not knl:
                for kt in range(nkt):
                    d = nc.dram_tensor(f"d_ec_{h}_{kt}", [P, 2, 512], BF16,
                                       kind="ExternalOutput")
                    nc.sync.dma_start(out=d.ap(), in_=elist[kt])
            for stt in range(S // P):
                for kt in range(nkt):
                    esl = efs[kt][:, stt * P:(stt + 1) * P]
                    first, last = kt == 0, kt == nkt - 1
                    nc.tensor.matmul(cmat[:, stt, :], lhsT=esl,
                                     rhs=vmat[:, kt, h, :],
                                     start=first, stop=last)
                    nc.tensor.matmul(den[:, int(knl), stt:stt + 1], lhsT=esl,
                                     rhs=ones2, start=first, stop=last)

        def flush_branch(h, knl):
            # reciprocal of the denominator, then a fused normalize that
            # reads ctx straight from PSUM (frees the bufs=1 accumulator)
            rc = cspool.tile([P, S // P, 1], F32,
                             name=f"rc_{h}_{int(knl)}",
                             tag=("rcK" if knl else "rcS"))
            nc.vector.reciprocal(
                rc, den[:, int(knl), :].unsqueeze(2))
            t = cspool.tile([P, S // P, HD], F32,
                            name=f"t_{h}_{int(knl)}",
                            tag=("t2" if knl else "t1"), bufs=3)
            nc.vector.tensor_tensor(
                out=t, in0=ctxK if knl else ctxS,
                in1=rc.broadcast_to([P, S // P, HD]), op=ALU.mult)
            if _DUMP and h in (0, 1, 6, 7):
                d = nc.dram_tensor(f"d_t_{h}_{int(knl)}", [P, S // P, HD],
                                   F32, kind="ExternalOutput")
                nc.sync.dma_start(out=d.ap(), in_=t)
                d2 = nc.dram_tensor(f"d_rc_{h}_{int(knl)}", [P, S // P, 1],
                                    F32, kind="ExternalOutput")
                nc.sync.dma_start(out=d2.ap(), in_=rc)
            return t, None

        def post(h, t1, _u1, t2, _u2):
            # SBUF-only: the one op gpsimd can take off DVE's plate
            ot = op_t[h // 2][:, :, h % 2, :]
            if h < NHL - 2:
                nc.gpsimd.tensor_tensor(out=ot, in0=t1, in1=t2, op=ALU.add)
                nc.sync.dma_start(
                    out=out.ap()[:, h * HD:(h + 1) * HD].rearrange(
                        "(st p) j -> p st j", p=P),
                    in_=op_t[h // 2][:, :, h % 2, :])
            elif h == NHL - 2:
                # h6: add only; its columns ride the h7 pair-DMAs below
                nc.gpsimd.tensor_tensor(out=ot, in0=t1, in1=t2, op=ALU.add)
            else:
                # h7 tail: add in st-halves, each followed by a contiguous
                # two-head (512B/row) half-DMA that pipelines with the next add
                hs_ = S // P // 2
                for half in range(2):
                    sl = slice(half * hs_, (half + 1) * hs_)
                    nc.vector.tensor_tensor(out=ot[:, sl, :], in0=t1[:, sl, :],
                                            in1=t2[:, sl, :], op=ALU.add)
                    nc.sync.dma_start(
                        out=out.ap()[half * 512:(half + 1) * 512,
                                     (h - 1) * HD:(h + 1) * HD].rearrange(
                            "(st p) j -> p st j", p=P),
                        in_=op_t[h // 2][:, sl, :, :].rearrange(
                            "p st hh d -> p st (hh d)"))

        # ---- main schedule ----
        # slots 0..8: self(h) at slot h (h<8); knl(h) at slot h+2 for h<=5,
        # knl(6)/knl(7) in slot 8. Projection chunks are placed to meet their
        # availability deadlines while keeping per-slot PE load >= the ACT
        # (exp) cadence; ctx units trail their exp by two attention units.
        # a chunk must be EMITTED before any unit/block that reads its output;
        # same-slot placement is only safe when the consumer unit comes later
        # in the slot than the chunk's pacing position
        chunks = {
            -1: [("q", 0, 0), ("k", 0, 0), ("q", 0, 1)],
            0: [("k", 0, 1), ("v", 0), ("v", 1), ("v", 2), ("v", 3),
                ("v", 4), ("v", 5), ("v", 6), ("v", 7)],
            1: [("q", 1, 0), ("q", 1, 1), ("k", 1, 0), ("k", 1, 1),
                ("kq", 0, 0), ("kq", 0, 1), ("kk", 0, 0)],
            2: [("kv", 0), ("kv", 1), ("kv", 2), ("kv", 3),
                ("kq", 1, 0), ("kq", 1, 1), ("kk", 1, 0)],
            3: [("q", 2, 0), ("q", 2, 1), ("k", 2, 0)],
            4: [("k", 2, 1), ("kq", 2, 0), ("kq", 2, 1)],
            5: [("kk", 2, 0), ("q", 3, 0), ("q", 3, 1), ("k", 3, 0),
                ("k", 3, 1)],
            6: [("kq", 3, 0), ("kq", 3, 1)],
            7: [("kk", 3, 0)],
        }

        def emit_chunk(c, use_psc=False):
            if c[0] in ("v", "kv"):
                proj_v_chunk(c[0], Vsb if c[0] == "v" else KVsb, c[1])
            else:
                nm, jt, sc = c
                dst = {"q": QT, "k": KT, "kq": KQT, "kk": KKT}[nm]
                proj_t_chunk(nm, dst, hsTt, jt, sc, use_psc=use_psc)

        warmup(_WARMUP)
        for c in chunks[-1]:
            emit_chunk(c, use_psc=True)

        branch_es = {}      # (h, knl) -> [e tiles in kt order]
        ready = []          # [h, knl, elist, nkt, age]
        flushed = {}        # (h, knl) -> (t, None)

        defer_q = []        # head-0 blocks run in slot 7's idle stretch

        def note_unit(h, kt, knl, e, nkt):
            es = branch_es.setdefault((h, knl), [])
            es.append(e)
            if len(es) == nkt:
                if h == 0:
                    defer_q.append([h, knl, es, nkt])
                else:
                    ready.append([h, knl, es, nkt, 0])

        def run_deferred():
            h, knl, elist, nkt = defer_q.pop(0)
            emit_ctx_block(h, knl, elist, nkt)
            flushed[(h, knl)] = flush_branch(h, knl)
            if knl:
                post(h, *flushed.pop((h, False)), *flushed.pop((h, True)))

        def drain_pend(min_age):
            while ready and ready[0][4] >= min_age:
                h, knl, elist, nkt, _ = ready.pop(0)
                emit_ctx_block(h, knl, elist, nkt)
                flushed[(h, knl)] = flush_branch(h, knl)
                if knl:
                    post(h, *flushed.pop((h, False)),
                         *flushed.pop((h, True)))

        def age():
            for ent in ready:
                ent[4] += 1

        for slot in range(9):
            units = []
            if slot < 8:
                units += [(slot, kt, False, TKS // P) for kt in range(8)]
            ku = []
            if 2 <= slot <= 7:
                ku = [(slot - 2, kt, True, TKK // P) for kt in range(4)]
            elif slot == 8:
                ku = [(h, kt, True, TKK // P) for h in (6, 7)
                      for kt in range(4)]
            if ku and units:
                merged = []
                si, ki = 0, 0
                for pos in range(len(units) + len(ku)):
                    if pos in (8, 9, 10, 11) and ki < len(ku):
                        merged.append(ku[ki]); ki += 1
                    elif si < len(units):
                        merged.append(units[si]); si += 1
                    else:
                        merged.append(ku[ki]); ki += 1
                units = merged
            else:
                units += ku
            cl = list(chunks.get(slot, []))
            ncu = len(units)
            for i, (h, kt, knl, nkt) in enumerate(units):
                e = score_unit(h, kt, knl)
                note_unit(h, kt, knl, e, nkt)
                age()
                # ctx blocks first: they sit between the previous chunk's
                # copy and the next chunk's matmuls (psj WAR latency)
                drain_pend(6)
                if slot == 7 and i in (1, 7) and defer_q:
                    run_deferred()
                want = (i + 1) * len(cl) // max(ncu, 1)
                while cl and len(chunks[slot]) - len(cl) < want:
                    emit_chunk(cl.pop(0))
            for c in cl:
                emit_chunk(c)
        drain_pend(0)

        if _DUMP:
            for nm, t in [("d_QT", QT), ("d_KT", KT), ("d_KQT", KQT),
                          ("d_KKT", KKT), ("d_Vsb", Vsb), ("d_KVsb", KVsb),
                          ("d_hsTt0", hsTt[0]), ("d_wv", wvt["v"])]:
                d = nc.dram_tensor(nm, list(t.shape), BF16,
                                   kind="ExternalOutput")
                nc.sync.dma_start(out=d.ap(), in_=t)
            for i in range(4):
                d = nc.dram_tensor(f"d_op{i}", list(op_t[i].shape), F32,
                                   kind="ExternalOutput")
                nc.sync.dma_start(out=d.ap(), in_=op_t[i])

    nc.finalize()
    return nc


def _get_nc():
    if "nc" not in _CACHE:
        _CACHE["nc"] = _build()
    return _CACHE["nc"]


def kernel(**inputs):
    inp = {k: np.asarray(v, dtype=np.float32) for k, v in inputs.items()}
    nc = _get_nc()
    bf = ml_dtypes.bfloat16

    B = 4
    hsT_b = {}
    for b in range(B):
        hsv = inp["hidden_states"][b].T.astype(bf)          # [H, S]
        hsv = np.ascontiguousarray(
            hsv.reshape(HC, P, S).transpose(1, 0, 2))       # [p, hc, s]
        ehv = inp["encoder_hidden_states"][b].T.astype(bf)
        ehv = np.ascontiguousarray(
            ehv.reshape(HC, P, TKK).transpose(1, 0, 2))
        hsT_b[b] = (np.ascontiguousarray(hsv[:, :, :S // 2]),
                    np.ascontiguousarray(hsv[:, :, S // 2:]), ehv)

    wt_hg = {}
    for hg in range(2):
        m = {}
        for nm in ["q", "k", "kq", "kk"]:
            w = inp[f"W{nm}"][:, hg * HG:(hg + 1) * HG]
            m[f"wt_{nm}"] = np.ascontiguousarray(
                w.reshape(HC, P, NJT, P).transpose(2, 1, 0, 3).astype(bf))
            m[f"bt_{nm}"] = np.ascontiguousarray(
                inp[f"b{nm}"][hg * HG:(hg + 1) * HG].reshape(NJT, P).T)
        for nm in ["v", "kv"]:
            w = inp[f"W{nm}"][:, hg * HG:(hg + 1) * HG]
            m[f"wv_{nm}"] = np.ascontiguousarray(
                w.reshape(HC, P, HG).transpose(1, 0, 2).astype(bf))
            m[f"bv_{nm}"] = np.ascontiguousarray(
                inp[f"b{nm}"][hg * HG:(hg + 1) * HG])
        wt_hg[hg] = m

    in_maps = []
    for core in range(8):
        b, hg = core // 2, core % 2
        h0, h1, eh = hsT_b[b]
        m = {
            "hsT0": h0, "hsT1": h1, "ehsT": eh,
            "mask": np.ascontiguousarray(
                inp["attention_mask"][b, 0, 0, :].reshape(TKS // P, P).T),
            "emask": np.ascontiguousarray(
                inp["encoder_attention_mask"][b, 0, 0, :]
                .reshape(TKK // P, P).T),
        }
        m.update(wt_hg[hg])
        in_maps.append(m)

    res = run_bass_kernel_spmd(nc, in_maps, core_ids=list(range(8)))

    outp = np.empty((B, S, H), np.float32)
    for core in range(8):
        b, hg = core // 2, core % 2
        outp[b, :, hg * HG:(hg + 1) * HG] = res.results[core]["out"]
    return outp
